# revision 1
# baseline (speedup 1.0000x reference)
"""Multi-head attention (B=4, S=2048, D=1024, H=16, Dh=64) on 8 trn2 NeuronCores.

Sharding: core c -> (batch b = c//2, head-group g = c%2 of 8 heads).
Each core computes q/k/v projections for its 8 heads and the full attention,
writing o[b, :, 512*g : 512*(g+1)].  No collectives needed: the output's
feature dim is just the concatenation of per-head outputs.

Layout strategy (per core):
  - Host pre-transposes X (seq-major -> D-major) so the contraction dim D
    lands on SBUF partitions without on-chip transposes, and casts to bf16.
  - Projections compute qT/kT in [dh, seq] orientation (lhsT = W k-tile,
    rhs = X.T k-tile) and v in natural [seq, dh] orientation.
  - Scores are computed TRANSPOSED (scoresT[sk, sq] = kT.T @ qT) so that the
    exp'd attention matrix A.T is already in the [sk-partition, sq-free]
    layout the PV matmul needs as its stationary operand -> no transposes.
  - Softmax denominators come for free from a mask column appended to V
    (o_psum column 64 = sum over valid sk of A.T), so no reductions.
  - k-masking: V rows beyond V_len are zeroed on host and the mask column is
    0 there, so invalid sk contribute nothing to numerator or denominator.
    exp is computed without max-subtraction (scores are O(+-10), safe in
    fp32) which matches softmax exactly up to rounding.
  - q-masking + normalization fused: out_tile = o_psum[:, :64] *
    (maskq / sum) as a per-partition scalar multiply.

The program is compiled for SQT/SKT = ceil(max(Q_len)/128), ceil(max(V_len)/128)
tiles (shared SPMD program across the 8 cores), so work scales with the
actual sequence lengths.  Per-core smaller lengths are handled by the masks.
"""

import math

import numpy as np
import ml_dtypes


def _ensure_paths():
    import sys
    try:
        import concourse  # noqa: F401
        return
    except ImportError:
        pass
    for p in ("/opt/trn_rl_repo", "/root/.axon_site/_ro/trn_rl_repo"):
        if p not in sys.path:
            sys.path.insert(0, p)
    import concourse  # noqa: F401


P = 128          # SBUF partitions
D = 1024         # model dim
DH = 64          # head dim
HL = 8           # heads per core
E = HL * DH      # per-core output feature width (512)
NCORES = 8

_PROG_CACHE = {}

# exposed for test.py profiling reruns
_last_nc = None
_last_in_maps = None


def _build_program(SQT, SKT):
    """Build + bacc-compile the shared SPMD program for given tile counts."""
    _ensure_paths()
    import concourse.bass as bass  # noqa: F401
    import concourse.tile as tile
    from concourse import bacc, mybir

    BF = mybir.dt.bfloat16
    F32 = mybir.dt.float32
    Exp = mybir.ActivationFunctionType.Exp

    SQ = SQT * P
    SK = SKT * P
    QC = math.ceil(SQ / 512)   # sq chunks for matmul free dim / psum banks
    KC = math.ceil(SK / 512)
    KT = D // P                # 8 contraction tiles

    nc = bacc.Bacc("TRN2", target_bir_lowering=False, debug=False,
                   num_devices=NCORES)

    xqt = nc.dram_tensor("xqt", [D, SQ], BF, kind="ExternalInput").ap()
    xkt = nc.dram_tensor("xkt", [D, SK], BF, kind="ExternalInput").ap()
    xvt = nc.dram_tensor("xvt", [D, SK], BF, kind="ExternalInput").ap()
    wq = nc.dram_tensor("wq", [D, E], BF, kind="ExternalInput").ap()
    wk = nc.dram_tensor("wk", [D, E], BF, kind="ExternalInput").ap()
    wv = nc.dram_tensor("wv", [D, E], BF, kind="ExternalInput").ap()
    maskq = nc.dram_tensor("maskq", [SQT, P], F32, kind="ExternalInput").ap()
    maskk8 = nc.dram_tensor("maskk8", [SK, HL, 1], BF, kind="ExternalInput").ap()
    identd = nc.dram_tensor("ident", [P, P], F32, kind="ExternalInput").ap()
    out = nc.dram_tensor("out", [2048, E], F32, kind="ExternalOutput").ap()

    xqt_r = xqt.rearrange("(k p) s -> p k s", p=P)
    xkt_r = xkt.rearrange("(k p) s -> p k s", p=P)
    xvt_r = xvt.rearrange("(k p) s -> p k s", p=P)

    # at (A.T) buffering: double-buffer if it fits in SBUF alongside the rest
    at_bytes = SKT * min(SQ, 1024) * 2
    fixed_bytes = (3 * 8 * E * 2        # weights
                   + 3 * 8 * 512 * 2    # x stream bufs (proj phase)
                   + 4 * SQ * 2 + 4 * SK * 2 + SKT * HL * 65 * 2  # qt/kt/v
                   + 2 * SQ * 4         # oT bufs
                   + 4096)              # misc
    at_bufs = 2 if fixed_bytes + 2 * at_bytes < 188 * 1024 else 1

    with tile.TileContext(nc) as tc:
        with tc.tile_pool(name="const", bufs=1) as const, \
             tc.tile_pool(name="persist", bufs=1) as persist, \
             tc.tile_pool(name="atp", bufs=at_bufs) as atp, \
             tc.tile_pool(name="otp", bufs=2) as otp, \
             tc.tile_pool(name="small", bufs=6) as small:

            wq_sb = const.tile([P, KT, E], BF, tag="wq")
            wk_sb = const.tile([P, KT, E], BF, tag="wk")
            wv_sb = const.tile([P, KT, E], BF, tag="wv")
            maskq_sb = const.tile([P, SQT], F32, tag="mq")
            ident = const.tile([P, P], F32, tag="ident")
            nc.sync.dma_start(out=wq_sb, in_=wq.rearrange("(k p) e -> p k e", p=P))
            nc.sync.dma_start(out=wk_sb, in_=wk.rearrange("(k p) e -> p k e", p=P))
            nc.sync.dma_start(out=wv_sb, in_=wv.rearrange("(k p) e -> p k e", p=P))
            nc.sync.dma_start(out=maskq_sb, in_=maskq.rearrange("t p -> p t"))
            nc.sync.dma_start(out=ident, in_=identd)

            qt_sb = persist.tile([P, 4, SQ], BF, tag="qt")
            kt_sb = persist.tile([P, 4, SK], BF, tag="kt")
            # per-head qT with the unused partition half zeroed, so QK can use
            # full [128,128] lhsT tiles (the HAM activity monitor appears to
            # ignore partial-array matmuls, leaving the PE clock throttled)
            qt_pad = persist.tile([P, HL, SQ], BF, tag="qtp")
            # V tiles padded at the tail so lhsT can be read [128,128] wide
            v_sb = persist.tile([P, SKT, HL * (DH + 1) + DH - 1], BF, tag="v")
            nc.vector.memset(qt_pad, 0.0)
            # tail pad of v_sb is read as lhsT columns for head 7; zero it so
            # uninitialized SBUF (possibly NaN bit patterns) never reaches PSUM
            nc.vector.memset(v_sb[:, :, HL * (DH + 1):], 0.0)

            # ---- projections (own pools so SBUF/PSUM are released after) ----
            with tc.tile_pool(name="xs", bufs=3) as xs, \
                 tc.tile_pool(name="psp", bufs=2, space="PSUM") as psp:
                # qT, kT in [dh, seq] orientation, head-pair-major
                for dst, x_r, w_sb, nchunks, stot in (
                    (qt_sb, xqt_r, wq_sb, QC, SQ),
                    (kt_sb, xkt_r, wk_sb, KC, SK),
                ):
                    for c in range(nchunks):
                        c0 = 512 * c
                        ncols = min(512, stot - c0)
                        xt = xs.tile([P, KT, 512], BF, tag="x")
                        nc.sync.dma_start(out=xt[:, :, :ncols],
                                          in_=x_r[:, :, c0:c0 + ncols])
                        for p4 in range(4):
                            ps = psp.tile([P, 512], F32, tag="proj")
                            for k in range(KT):
                                nc.tensor.matmul(
                                    ps[:, :ncols],
                                    w_sb[:, k, P * p4:P * (p4 + 1)],
                                    xt[:, k, :ncols],
                                    start=(k == 0), stop=(k == KT - 1))
                            nc.vector.tensor_copy(out=dst[:, p4, c0:c0 + ncols],
                                                  in_=ps[:, :ncols])

                # v in natural [seq, dh] orientation + mask column
                for m in range(SKT):
                    xt = xs.tile([P, KT, 512], BF, tag="x")
                    nc.sync.dma_start(out=xt[:, :, :P],
                                      in_=xvt_r[:, :, P * m:P * (m + 1)])
                    ps = psp.tile([P, 512], F32, tag="proj")
                    for k in range(KT):
                        nc.tensor.matmul(ps, xt[:, k, :P], wv_sb[:, k, :],
                                         start=(k == 0), stop=(k == KT - 1))
                    v_m = v_sb[:, m, 0:HL * (DH + 1)].rearrange(
                        "p (h c) -> p h c", c=DH + 1)
                    nc.vector.tensor_copy(
                        out=v_m[:, :, 0:DH],
                        in_=ps.rearrange("p (h d) -> p h d", h=HL))
                    nc.sync.dma_start(out=v_m[:, :, DH:DH + 1],
                                      in_=maskk8[P * m:P * (m + 1)])

                # scatter qT halves into per-head zero-padded buffers
                # (partition-aligned SBUF->SBUF copies; even heads occupy
                # partitions 0:64 matching their kT rows, odd heads 64:128)
                for p4 in range(4):
                    nc.sync.dma_start(out=qt_pad[0:DH, 2 * p4, :],
                                      in_=qt_sb[0:DH, p4, :])
                    nc.sync.dma_start(out=qt_pad[DH:P, 2 * p4 + 1, :],
                                      in_=qt_sb[DH:P, p4, :])

            # ---- attention: software-pipelined, sq split into passes ----
            # The PE executes in program order; with a single score-psum the
            # chain QK(t) -> exp(t) -> QK(t+1) serializes PE behind ACT and
            # the HAM clock gate never warms.  Splitting sq into <=1024-wide
            # passes makes the score psum 2 banks, so it double-buffers
            # (bufs=2), and PV/transpose work of the previous (head, pass) is
            # emitted before each QK step to keep the in-order PE stream dense.
            chunk_list = []
            off = 0
            while off < SQ:
                n = min(512, SQ - off)
                chunk_list.append((off, n))
                off += n
            passes = []  # (pass_off, [chunk sizes]) with total <= 1024
            cur = []
            cur_off = 0
            for (co, n) in chunk_list:
                if sum(cur) + n > 1024 and cur:
                    passes.append((cur_off, cur))
                    cur = []
                    cur_off = co
                cur.append(n)
            passes.append((cur_off, cur))

            with tc.tile_pool(name="psq", bufs=2, space="PSUM") as psq, \
                 tc.tile_pool(name="psot", bufs=2, space="PSUM") as psot, \
                 tc.tile_pool(name="pstr", bufs=2, space="PSUM") as pstr:

              def emit_qk_step(h, t, at, poff, csizes, psz):
                  p4 = h // 2
                  ps = psq.tile([P, 1024], F32, tag="qk",
                                name=f"qk_{h}_{t}_{poff}")
                  c0 = 0
                  for n in csizes:
                      nc.tensor.matmul(
                          ps[:, c0:c0 + n],
                          kt_sb[:, p4, P * t:P * (t + 1)],
                          qt_pad[:, h, poff + c0:poff + c0 + n],
                          start=True, stop=True)
                      c0 += n
                  nc.scalar.activation(out=at[:, t, :psz], in_=ps[:, :psz],
                                       func=Exp, scale=0.125)

              def pv_pieces(h, at, poff, csizes, psz):
                  """Closures emitting PV + transpose + normalize for one
                  (head, pass)."""
                  ot = otp.tile([P, 1024], F32, tag="ot",
                                name=f"ot_{h}_{poff}")
                  pieces = []

                  def mk_pv(c0, n, t0, t1, po_box):
                      def go():
                          if t0 == 0:
                              po_box[0] = psot.tile([P, 512], F32, tag="o",
                                                    name=f"po_{h}_{poff}_{c0}")
                          po = po_box[0]
                          h0 = h * (DH + 1)
                          for t in range(t0, t1):
                              nc.tensor.matmul(po[:, :n],
                                               v_sb[:, t, h0:h0 + P],
                                               at[:, t, c0:c0 + n],
                                               start=(t == 0),
                                               stop=(t == SKT - 1))
                          if t1 == SKT:
                              nc.vector.tensor_copy(out=ot[:, c0:c0 + n],
                                                    in_=po[:, :n])
                      return go

                  c0 = 0
                  for n in csizes:
                      box = [None]
                      if SKT > 6:
                          half_t = (SKT + 1) // 2
                          pieces.append(mk_pv(c0, n, 0, half_t, box))
                          pieces.append(mk_pv(c0, n, half_t, SKT, box))
                      else:
                          pieces.append(mk_pv(c0, n, 0, SKT, box))
                      c0 += n

                  def mk_tr(lsq, gsq):
                      def go():
                          po2 = pstr.tile([P, P], F32, tag="tr",
                                          name=f"po2_{h}_{gsq}")
                          # transpose as a REGULAR full-array fp32 matmul
                          # (ot.T @ I); identity rows >= 65 are zero so the
                          # junk rows of ot never reach the output
                          nc.tensor.matmul(po2, ot[:, P * lsq:P * (lsq + 1)],
                                           ident, start=True, stop=True)
                          rc = small.tile([P, 1], F32, tag="rc",
                                          name=f"rc_{h}_{gsq}")
                          sc = small.tile([P, 1], F32, tag="sc",
                                          name=f"sc_{h}_{gsq}")
                          nc.vector.reciprocal(rc, po2[:, DH:DH + 1])
                          nc.vector.tensor_mul(sc, rc, maskq_sb[:, gsq:gsq + 1])
                          ob = small.tile([P, DH], F32, tag="ob",
                                          name=f"ob_{h}_{gsq}")
                          nc.vector.tensor_scalar_mul(ob, po2[:, 0:DH], sc)
                          nc.sync.dma_start(
                              out=out[P * gsq:P * (gsq + 1),
                                      DH * h:DH * (h + 1)],
                              in_=ob)
                      return go

                  for lsq in range(psz // P):
                      pieces.append(mk_tr(lsq, poff // P + lsq))
                  return pieces

              prev_pieces = []
              for poff, csizes in passes:
                  psz = sum(csizes)
                  for h in range(HL):
                      at = atp.tile([P, SKT, psz], BF, tag="at",
                                    name=f"at_{h}_{poff}")
                      L = len(prev_pieces)
                      done = 0
                      for t in range(SKT):
                          upto = (L * (t + 1)) // SKT
                          while done < upto:
                              prev_pieces[done]()
                              done += 1
                          emit_qk_step(h, t, at, poff, csizes, psz)
                      while done < L:
                          prev_pieces[done]()
                          done += 1
                      prev_pieces = pv_pieces(h, at, poff, csizes, psz)
              for piece in prev_pieces:
                  piece()

    nc.compile()
    return nc


def _get_program(SQT, SKT):
    key = (SQT, SKT)
    if key not in _PROG_CACHE:
        _PROG_CACHE[key] = _build_program(SQT, SKT)
    return _PROG_CACHE[key]


def _ident128():
    i = np.zeros((P, P), dtype=np.float32)
    for k in range(DH + 1):
        i[k, k] = 1.0
    return i


def kernel(Q_seq, K_seq, V_seq, WQ, WK, WV, Q_len, V_len):
    global _last_nc, _last_in_maps
    _ensure_paths()
    from concourse.bass_utils import run_bass_kernel_spmd

    Q_seq = np.asarray(Q_seq, dtype=np.float32)
    K_seq = np.asarray(K_seq, dtype=np.float32)
    V_seq = np.asarray(V_seq, dtype=np.float32)
    WQ = np.asarray(WQ, dtype=np.float32)
    WK = np.asarray(WK, dtype=np.float32)
    WV = np.asarray(WV, dtype=np.float32)
    Q_len = np.asarray(Q_len).reshape(-1)
    V_len = np.asarray(V_len).reshape(-1)

    B, S, _ = Q_seq.shape
    BF = ml_dtypes.bfloat16

    SQT = max(1, math.ceil(int(Q_len.max()) / P))
    SKT = max(1, math.ceil(int(V_len.max()) / P))
    SQ, SK = SQT * P, SKT * P

    nc = _get_program(SQT, SKT)

    in_maps = []
    for c in range(NCORES):
        b, g = c // 2, c % 2
        ql, vl = int(Q_len[b]), int(V_len[b])
        mk = (np.arange(SK) < vl)
        xq = np.ascontiguousarray(Q_seq[b, :SQ].T).astype(BF)
        xk = np.ascontiguousarray(K_seq[b, :SK].T).astype(BF)
        xv = np.ascontiguousarray((V_seq[b, :SK] * mk[:, None]).T).astype(BF)
        in_maps.append({
            "xqt": xq,
            "xkt": xk,
            "xvt": xv,
            "wq": np.ascontiguousarray(WQ[:, E * g:E * (g + 1)]).astype(BF),
            "wk": np.ascontiguousarray(WK[:, E * g:E * (g + 1)]).astype(BF),
            "wv": np.ascontiguousarray(WV[:, E * g:E * (g + 1)]).astype(BF),
            "maskq": (np.arange(SQ) < ql).astype(np.float32).reshape(SQT, P),
            "maskk8": np.repeat(mk.astype(BF)[:, None], HL, axis=1)[..., None],
            "ident": _ident128(),
        })

    res = run_bass_kernel_spmd(nc, in_maps, core_ids=list(range(NCORES)))
    _last_nc, _last_in_maps = nc, in_maps

    full = np.zeros((B, S, 2 * E), dtype=np.float32)
    for c in range(NCORES):
        b, g = c // 2, c % 2
        o = res.results[c]["out"]
        # rows >= SQ are never written by the kernel; keep host zeros there
        full[b, :SQ, E * g:E * (g + 1)] = o[:SQ]
    return full



# revision 3
# speedup vs baseline: 2.0405x; 2.0405x over previous
"""Multi-head attention (B=4, S=2048, D=1024, H=16, Dh=64) on 8 trn2 NeuronCores.

Sharding: core c -> heads (2c, 2c+1) of ALL 4 batches.  Every batch has 16
heads, so each core gets exactly 2 heads x 4 batches and per-core attention
work is Sum_b SQT_b*SKT_b score tiles -- perfectly balanced across cores
regardless of the per-batch sequence lengths (the old batch-sharded layout
made the largest-batch core ~2.1x slower than the mean).

Per core (2 heads, head A on partitions 0:64, head B on 64:128):
  - Host pre-transposes X per batch (D-major) in bf16 and concatenates the
    batches along seq: xq [D, SQtot], xk/xv [D, SKtot] (V rows >= V_len are
    zeroed on host).
  - Projections: qT/kT in [dh, seq] orientation, v in natural [seq, dh]
    orientation with a mask column appended per head (denominator trick).
  - QK computes scoresT[sk, sq] with K=64 contraction, the two heads issued
    back-to-back to complementary row groups (tile_position (0,0)/(64,0)) so
    they run concurrently in the PE array -- 2x QK throughput vs zero-padding
    the contraction to 128.
  - exp on ScalarE in groups of up to 3 sk-tiles (one 3-bank PSUM tile per
    group) to amortize the ~293ns fixed ACTIVATE overhead.
  - PV accumulates oT[65, sq] per head (row 64 = softmax denominator via the
    mask column).  NO on-device transpose or normalization: the kernel ships
    oT + denominator to DRAM and the HOST does o = (num/den).T and the
    Q_len row masking during unsharding.  This removes the fp32 PE-transpose
    matmuls (~80us of PE time in the old kernel) entirely.
  - Emission is software-pipelined: the next batch's projection pieces and
    the previous chunk's PV pieces are interleaved between QK groups to keep
    the in-order PE queue dense while ScalarE (the attention-phase
    bottleneck) drains the exp queue.

The program is compiled for the runtime tile counts (SQT_b, SKT_b) =
ceil(len/128) per batch (shared SPMD program across the 8 cores).
"""

import math

import numpy as np
import ml_dtypes


def _ensure_paths():
    import sys
    try:
        import concourse  # noqa: F401
        return
    except ImportError:
        pass
    for p in ("/opt/trn_rl_repo", "/root/.axon_site/_ro/trn_rl_repo"):
        if p not in sys.path:
            sys.path.insert(0, p)
    import concourse  # noqa: F401


P = 128          # SBUF partitions
D = 1024         # model dim
DH = 64          # head dim
KT = D // P      # contraction tiles for projections
GN = 3           # sk-tiles per exp group (3 PSUM banks)
NB = 4           # batches
NCORES = 8

_PROG_CACHE = {}

# exposed for test.py profiling reruns
_last_nc = None
_last_in_maps = None


def _chunks(total, sz=512):
    out = []
    o = 0
    while o < total:
        n = min(sz, total - o)
        out.append((o, n))
        o += n
    return out


def _build_program(SQT, SKT):
    """Build + bacc-compile the shared SPMD program for given per-batch tile
    counts (SQT, SKT are 4-tuples)."""
    _ensure_paths()
    import concourse.bass as bass  # noqa: F401
    import concourse.tile as tile
    from concourse import bacc, mybir

    BF = mybir.dt.bfloat16
    F32 = mybir.dt.float32
    Exp = mybir.ActivationFunctionType.Exp

    SQ = [t * P for t in SQT]
    SK = [t * P for t in SKT]
    SQtot = sum(SQ)
    SKtot = sum(SK)
    QOFF = [sum(SQ[:b]) for b in range(NB)]
    KOFF = [sum(SK[:b]) for b in range(NB)]
    TOFF = [sum(SKT[:b]) for b in range(NB)]
    SKTtot = sum(SKT)
    ATM = max(SKT)

    nc = bacc.Bacc("TRN2", target_bir_lowering=False, debug=False,
                   num_devices=NCORES)

    xq = nc.dram_tensor("xq", [D, SQtot], BF, kind="ExternalInput").ap()
    xk = nc.dram_tensor("xk", [D, SKtot], BF, kind="ExternalInput").ap()
    xv = nc.dram_tensor("xv", [D, SKtot], BF, kind="ExternalInput").ap()
    wq = nc.dram_tensor("wq", [D, P], BF, kind="ExternalInput").ap()
    wk = nc.dram_tensor("wk", [D, P], BF, kind="ExternalInput").ap()
    wv = nc.dram_tensor("wv", [D, P], BF, kind="ExternalInput").ap()
    mk2 = nc.dram_tensor("mk2", [SKtot, 2, 1], BF, kind="ExternalInput").ap()
    out = nc.dram_tensor("out", [2, DH + 1, SQtot], F32,
                         kind="ExternalOutput").ap()

    xq_r = xq.rearrange("(k p) s -> p k s", p=P)
    xk_r = xk.rearrange("(k p) s -> p k s", p=P)
    xv_r = xv.rearrange("(k p) s -> p k s", p=P)

    VW = 2 * (DH + 1)        # 130: [A num 64 | A mask | B num 64 | B mask]
    VWP = VW + DH - 1        # padded so lhsT for head B reads 128 cols

    with tile.TileContext(nc) as tc:
        with tc.tile_pool(name="const", bufs=1) as const, \
             tc.tile_pool(name="persist", bufs=1) as persist, \
             tc.tile_pool(name="xs", bufs=3) as xs, \
             tc.tile_pool(name="atp", bufs=2) as atp, \
             tc.tile_pool(name="otp", bufs=2) as otp, \
             tc.tile_pool(name="psq", bufs=2, space="PSUM") as psq, \
             tc.tile_pool(name="pss", bufs=2, space="PSUM") as pss:

            wq_sb = const.tile([P, KT, P], BF, tag="wq")
            wk_sb = const.tile([P, KT, P], BF, tag="wk")
            wv_sb = const.tile([P, KT, P], BF, tag="wv")
            nc.sync.dma_start(out=wq_sb, in_=wq.rearrange("(k p) e -> p k e", p=P))
            nc.sync.dma_start(out=wk_sb, in_=wk.rearrange("(k p) e -> p k e", p=P))
            nc.sync.dma_start(out=wv_sb, in_=wv.rearrange("(k p) e -> p k e", p=P))

            qt = persist.tile([P, SQtot], BF, tag="qt")
            kt = persist.tile([P, SKtot], BF, tag="kt")
            v_sb = persist.tile([P, SKTtot, VWP], BF, tag="v")
            # tail pad is read as lhsT columns for head B; zero it so
            # uninitialized SBUF never reaches PSUM
            nc.vector.memset(v_sb[:, :, VW:], 0.0)

            # ---------------- projection pieces (closures) ----------------
            def q_proj_piece(b, c0, n, dst, src_r, w_sb, off):
                def go():
                    xt = xs.tile([P, KT, 512], BF, tag="x")
                    nc.sync.dma_start(out=xt[:, :, :n],
                                      in_=src_r[:, :, off + c0:off + c0 + n])
                    ps = pss.tile([P, 512], F32, tag="acc")
                    for k in range(KT):
                        nc.tensor.matmul(ps[:, :n], w_sb[:, k, :], xt[:, k, :n],
                                         start=(k == 0), stop=(k == KT - 1))
                    nc.vector.tensor_copy(out=dst[:, off + c0:off + c0 + n],
                                          in_=ps[:, :n])
                return go

            def v_proj_piece(b, c0, n):
                def go():
                    xt = xs.tile([P, KT, 512], BF, tag="x")
                    nc.sync.dma_start(
                        out=xt[:, :, :n],
                        in_=xv_r[:, :, KOFF[b] + c0:KOFF[b] + c0 + n])
                    nt = n // P
                    ta = TOFF[b] + c0 // P
                    for st in range(nt):
                        ps = pss.tile([P, 512], F32, tag="acc")
                        for k in range(KT):
                            nc.tensor.matmul(
                                ps[:, :P], xt[:, k, P * st:P * (st + 1)],
                                wv_sb[:, k, :],
                                start=(k == 0), stop=(k == KT - 1))
                        vt = v_sb[:, ta + st, 0:VW].rearrange(
                            "p (g c) -> p g c", c=DH + 1)
                        nc.vector.tensor_copy(
                            out=vt[:, :, 0:DH],
                            in_=ps[:, :P].rearrange("p (g c) -> p g c", c=DH))
                        a0 = KOFF[b] + c0 + P * st
                        nc.sync.dma_start(out=vt[:, :, DH:DH + 1],
                                          in_=mk2[a0:a0 + P])
                return go

            def proj_pieces(b):
                ps_ = []
                for c0, n in _chunks(SK[b]):
                    ps_.append(q_proj_piece(b, c0, n, kt, xk_r, wk_sb, KOFF[b]))
                for c0, n in _chunks(SK[b]):
                    ps_.append(v_proj_piece(b, c0, n))
                for c0, n in _chunks(SQ[b]):
                    ps_.append(q_proj_piece(b, c0, n, qt, xq_r, wq_sb, QOFF[b]))
                return ps_

            # ---------------- attention ----------------
            def pv_pieces(b, c0, n, ats):
                """PV + evac + output-DMA closures for one finished chunk."""
                pieces = []

                def mk_pv(g, t0, t1, po_box):
                    def go():
                        if t0 == 0:
                            po_box[0] = pss.tile([P, 512], F32, tag="acc",
                                                 name=f"po_{b}_{c0}_{g}")
                        po = po_box[0]
                        for t in range(t0, t1):
                            nc.tensor.matmul(
                                po[:, :n],
                                v_sb[:, TOFF[b] + t, (DH + 1) * g:
                                     (DH + 1) * g + P],
                                ats[g][:, t, :n],
                                start=(t == 0), stop=(t == SKT[b] - 1))
                        if t1 == SKT[b]:
                            ot = otp.tile([DH + 1, 512], F32, tag="ot",
                                          name=f"ot_{b}_{c0}_{g}")
                            nc.vector.tensor_copy(out=ot[:, :n],
                                                  in_=po[0:DH + 1, :n])
                            nc.sync.dma_start(
                                out=out[g, :, QOFF[b] + c0:QOFF[b] + c0 + n],
                                in_=ot[:, :n])
                    return go

                for g in range(2):
                    box = [None]
                    if SKT[b] > 6:
                        half = (SKT[b] + 1) // 2
                        pieces.append(mk_pv(g, 0, half, box))
                        pieces.append(mk_pv(g, half, SKT[b], box))
                    else:
                        pieces.append(mk_pv(g, 0, SKT[b], box))
                return pieces

            pend = []          # side-work: prev-chunk PV + next-batch proj

            def emit_chunk(b, c0, n):
                nonlocal pend
                ats = (atp.tile([P, ATM, 512], BF, tag="ata",
                                name=f"ata_{b}_{c0}"),
                       atp.tile([P, ATM, 512], BF, tag="atb",
                                name=f"atb_{b}_{c0}"))
                groups = [(t0, min(GN, SKT[b] - t0))
                          for t0 in range(0, SKT[b], GN)]
                side = pend
                pend = []
                L = len(side)
                done = 0
                for gi, (t0, gn) in enumerate(groups):
                    pq = [psq.tile([P, GN, 512], F32, tag="qk",
                                   name=f"qk_{b}_{c0}_{t0}_{g}")
                          for g in range(2)]
                    for j in range(gn):
                        t = t0 + j
                        for g in range(2):
                            nc.tensor.matmul(
                                pq[g][:, j, :n],
                                kt[DH * g:DH * (g + 1),
                                   KOFF[b] + P * t:KOFF[b] + P * (t + 1)],
                                qt[DH * g:DH * (g + 1),
                                   QOFF[b] + c0:QOFF[b] + c0 + n],
                                start=True, stop=True)
                    for g in range(2):
                        nc.scalar.activation(
                            out=ats[g][:, t0:t0 + gn, :n],
                            in_=pq[g][:, 0:gn, :n],
                            func=Exp, scale=0.125)
                    upto = (L * (gi + 1)) // len(groups)
                    while done < upto:
                        side[done]()
                        done += 1
                while done < L:
                    side[done]()
                    done += 1
                pend = pv_pieces(b, c0, n, ats)

            # b0 projections emitted directly (nothing to overlap them with)
            for piece in proj_pieces(0):
                piece()

            for b in range(NB):
                filler = proj_pieces(b + 1) if b + 1 < NB else []
                ch = _chunks(SQ[b])
                # ration the next batch's proj pieces across this batch's
                # chunks (prepended to pend so they emit before PV waits)
                nf = len(filler)
                fdone = 0
                for ci, (c0, n) in enumerate(ch):
                    take = (nf * (ci + 1)) // len(ch) - fdone
                    pend = filler[fdone:fdone + take] + pend
                    fdone += take
                    emit_chunk(b, c0, n)
            for piece in pend:
                piece()

    nc.compile()
    return nc


def _get_program(SQT, SKT):
    key = (tuple(SQT), tuple(SKT))
    if key not in _PROG_CACHE:
        _PROG_CACHE[key] = _build_program(key[0], key[1])
    return _PROG_CACHE[key]


def _prep_inputs(Q_seq, K_seq, V_seq, WQ, WK, WV, Q_len, V_len):
    """Host-side shared prep: per-batch transposed bf16 activations and
    masks, concatenated along seq; returns (SQT, SKT, shared dict)."""
    BF = ml_dtypes.bfloat16
    B = Q_seq.shape[0]
    SQT = [max(1, math.ceil(int(Q_len[b]) / P)) for b in range(B)]
    SKT = [max(1, math.ceil(int(V_len[b]) / P)) for b in range(B)]
    SQ = [t * P for t in SQT]
    SK = [t * P for t in SKT]

    xq = np.concatenate(
        [np.ascontiguousarray(Q_seq[b, :SQ[b]].T) for b in range(B)],
        axis=1).astype(BF)
    xk = np.concatenate(
        [np.ascontiguousarray(K_seq[b, :SK[b]].T) for b in range(B)],
        axis=1).astype(BF)
    mks = [(np.arange(SK[b]) < int(V_len[b])) for b in range(B)]
    xv = np.concatenate(
        [np.ascontiguousarray((V_seq[b, :SK[b]] * mks[b][:, None]).T)
         for b in range(B)], axis=1).astype(BF)
    mk2 = np.concatenate(mks)[:, None, None].astype(BF)
    mk2 = np.repeat(mk2, 2, axis=1)
    return SQT, SKT, {"xq": xq, "xk": xk, "xv": xv, "mk2": mk2}


def kernel(Q_seq, K_seq, V_seq, WQ, WK, WV, Q_len, V_len):
    global _last_nc, _last_in_maps
    _ensure_paths()
    from concourse.bass_utils import run_bass_kernel_spmd

    Q_seq = np.asarray(Q_seq, dtype=np.float32)
    K_seq = np.asarray(K_seq, dtype=np.float32)
    V_seq = np.asarray(V_seq, dtype=np.float32)
    WQ = np.asarray(WQ, dtype=np.float32)
    WK = np.asarray(WK, dtype=np.float32)
    WV = np.asarray(WV, dtype=np.float32)
    Q_len = np.asarray(Q_len).reshape(-1)
    V_len = np.asarray(V_len).reshape(-1)

    B, S, _ = Q_seq.shape
    BF = ml_dtypes.bfloat16

    SQT, SKT, shared = _prep_inputs(Q_seq, K_seq, V_seq, WQ, WK, WV,
                                    Q_len, V_len)
    SQ = [t * P for t in SQT]
    QOFF = [sum(SQ[:b]) for b in range(B)]

    nc = _get_program(SQT, SKT)

    in_maps = []
    for c in range(NCORES):
        m = dict(shared)
        m["wq"] = np.ascontiguousarray(WQ[:, P * c:P * (c + 1)]).astype(BF)
        m["wk"] = np.ascontiguousarray(WK[:, P * c:P * (c + 1)]).astype(BF)
        m["wv"] = np.ascontiguousarray(WV[:, P * c:P * (c + 1)]).astype(BF)
        in_maps.append(m)

    res = run_bass_kernel_spmd(nc, in_maps, core_ids=list(range(NCORES)))
    _last_nc, _last_in_maps = nc, in_maps

    H = 16
    full = np.zeros((B, S, H * DH), dtype=np.float32)
    for c in range(NCORES):
        o = res.results[c]["out"]          # [2, 65, SQtot]
        for g in range(2):
            h = 2 * c + g
            num = o[g, :DH]                # [64, SQtot]
            den = o[g, DH:DH + 1]          # [1, SQtot]
            ot = num / den
            for b in range(B):
                ql = int(Q_len[b])
                sl = ot[:, QOFF[b]:QOFF[b] + SQ[b]]
                full[b, :SQ[b], DH * h:DH * (h + 1)] = sl.T
                full[b, ql:, DH * h:DH * (h + 1)] = 0.0
    return full
